# revision 11
# baseline (speedup 1.0000x reference)
"""MLA (multi-head latent attention) prefill kernel for 8 Trainium2 NeuronCores.

Sharding: pure data-parallel over (batch, query-chunk). Core c handles batch
c//4 and query rows [512*(c%4), 512*(c%4+1)). Keys/KV-latent (rank 512) are
computed per-core from the full hidden states of the batch (replicated compute,
~9% FLOP overhead) so there are ZERO collectives; every core writes a disjoint
[512, 2048] slice of the output.

All matmuls run in bf16 (full PE rate + fast-weight-load so LDWEIGHTS hides
under the previous matmul's stream), accumulating in fp32 PSUM. LayerNorm
gains/biases are folded into the downstream weights on the host (the q-score
bias shift cancels in softmax; the value-path bias uses sum(probs)=1), so the
device LN is a single (x-mean)*rstd tensor_scalar op applied directly to the
PSUM accumulator. Softmax skips the max-subtraction (logits are O(1): exp
never overflows) and the denominator is an all-ones matmul on the PE. The
o_proj weight is prefetched into SBUF over the scalar-engine DMA ring; attn_v
results stay resident in SBUF (no DRAM round-trip). DMA issue order is tuned
so the first matmul's inputs (hsq, wqa) lead the sync ring.
"""

import math
from contextlib import ExitStack

import ml_dtypes
import numpy as np

import concourse.bass as bass
import concourse.tile as tile
from concourse import bacc, mybir
from concourse.bass_utils import run_bass_kernel_spmd
from concourse.masks import make_identity

F32 = mybir.dt.float32
BF16 = mybir.dt.bfloat16
AF = mybir.ActivationFunctionType
OP = mybir.AluOpType

# problem dims (hardcoded per contest contract)
B, S, D = 2, 2048, 2048
H = 16
LAT = 1536          # Q_LORA
R = 512             # KV_LORA
DN, DR, DV = 128, 64, 128
EPS = 1e-5
SCALE = 1.0 / math.sqrt(DN + DR)

P = 128
CH = 512            # queries per core
NQT = CH // P       # 4 q-tiles per core
NKT = S // P        # 16 key tiles
NDT = D // P        # 16 model-dim tiles
NLT = LAT // P      # 12 latent tiles

N_CORES = 8


def _bcast_rows(t, n, length):
    """[length] DRAM vector -> [n, length] AP with partition step 0."""
    return bass.AP(tensor=t, offset=0, ap=[[0, n], [1, length]])


def build_nc():
    nc = bacc.Bacc(None, target_bir_lowering=False)

    # ---- DRAM I/O (per-core shapes; bf16 where fed to the PE) ----
    hst4 = nc.dram_tensor("hst4", [NDT, NKT, P, P], BF16, kind="ExternalInput")
    hsq4 = nc.dram_tensor("hsq4", [NQT, P, NDT, P], BF16, kind="ExternalInput")
    wqa_t = nc.dram_tensor("wqa_t", [D, LAT], BF16, kind="ExternalInput")
    wqb_t = nc.dram_tensor("wqb_t", [LAT, H * DN], BF16, kind="ExternalInput")
    wqr_t = nc.dram_tensor("wqr_t", [LAT, H * DR], BF16, kind="ExternalInput")
    wkva_t = nc.dram_tensor("wkva_t", [D, R + DR], BF16, kind="ExternalInput")
    kup = nc.dram_tensor("kup", [H * DN, R], BF16, kind="ExternalInput")
    vup = nc.dram_tensor("vup", [H, P, NQT, P], BF16, kind="ExternalInput")
    wo_t = nc.dram_tensor("wo_t", [H * DV, D], BF16, kind="ExternalInput")
    qb_bias = nc.dram_tensor("qb_bias", [H * DN], F32, kind="ExternalInput")
    qr_bias = nc.dram_tensor("qr_bias", [H * DR], F32, kind="ExternalInput")
    av_bias = nc.dram_tensor("av_bias", [H * DV], F32, kind="ExternalInput")
    ck_tab = nc.dram_tensor("ck_tab", [S, DR // 2], F32, kind="ExternalInput")
    sk_tab = nc.dram_tensor("sk_tab", [S, DR // 2], F32, kind="ExternalInput")
    cq_tab = nc.dram_tensor("cq_tab", [P, CH], F32, kind="ExternalInput")
    sq_tab = nc.dram_tensor("sq_tab", [P, CH], F32, kind="ExternalInput")
    out_c = nc.dram_tensor("out_c", [CH, D], F32, kind="ExternalOutput")

    with tile.TileContext(nc) as tc, ExitStack() as octx:
        res = octx.enter_context(tc.tile_pool(name="res", bufs=1))
        # k_full^T: 5 c-subtiles (4x128 latent + 64 rope) x 2048 keys
        kfull = res.tile([P, 5, S], BF16)
        # kv latent, natural layout: [key-part, keytile, R]
        kvlat = res.tile([P, NKT, R], BF16)
        # q latent transposed: [lat-part, lat-tile, q]
        qlat_t = res.tile([P, NLT, CH], BF16)
        # attn_v outputs for all heads, resident until o_proj
        avn = res.tile([P, H, CH], BF16)
        # o_proj weight, resident (DMA issued after phase-1a emission)
        wo_all = res.tile([P, H, D], BF16)

        consts = octx.enter_context(tc.tile_pool(name="consts", bufs=1))
        ident = consts.tile([P, P], BF16)
        make_identity(nc, ident)
        ones_t = consts.tile([P, P], BF16)
        nc.vector.memset(ones_t, 1.0)
        eps_t = consts.tile([P, 1], F32)
        nc.vector.memset(eps_t, EPS)
        cq_t = consts.tile([P, CH], F32)
        sq_t = consts.tile([P, CH], F32)
        qbb = consts.tile([P, H], F32)
        qrb = consts.tile([P, 8], F32)
        avb = consts.tile([P, H], F32)

        with ExitStack() as p01:
            wkvp = p01.enter_context(tc.tile_pool(name="wkvp", bufs=1))
            wkv_all = wkvp.tile([P, NDT, R + DR], BF16)
            gbkv = p01.enter_context(tc.tile_pool(name="gbkv", bufs=1))
            ck_t = gbkv.tile([P, NKT, DR // 2], F32)
            sk_t = gbkv.tile([P, NKT, DR // 2], F32)

            # ================= phase 1a: q latent (LN(hs_q @ w_qa.T))^T =============
            with ExitStack() as p1:
                hsqp = p1.enter_context(tc.tile_pool(name="hsqp", bufs=1))
                wqap = p1.enter_context(tc.tile_pool(name="wqap", bufs=4))
                mixp = p1.enter_context(tc.tile_pool(name="mixp", bufs=1))
                lnp = p1.enter_context(tc.tile_pool(name="lnp", bufs=2))
                psq = p1.enter_context(tc.tile_pool(name="psq", bufs=1, space="PSUM"))

                # hsq first on the sync ring (first matmul's stationary operand)
                hsq_all = hsqp.tile([P, NQT, NDT, P], BF16)
                for qt in range(NQT):
                    nc.sync.dma_start(
                        hsq_all[:, qt], hsq4[qt].rearrange("p d c -> p d c")
                    )
                qmix_all = mixp.tile([P, NQT, LAT], BF16)

                for j in range(3):
                    pqs = []
                    for qt in range(NQT):
                        pq = psq.tile([P, 512], F32, tag=f"pq{qt}", bufs=1, name=f"pq{qt}")
                        pqs.append(pq)
                    for dt in range(NDT):
                        wqa_c = wqap.tile([P, 512], BF16, tag="wqa")
                        nc.sync.dma_start(
                            wqa_c[:],
                            wqa_t[dt * P:(dt + 1) * P, j * 512:(j + 1) * 512],
                        )
                        for qt in range(NQT):
                            nc.tensor.matmul(
                                pqs[qt][:], hsq_all[:, qt, dt, :], wqa_c[:],
                                start=(dt == 0), stop=(dt == NDT - 1),
                            )
                    for qt in range(NQT):
                        nc.vector.tensor_copy(
                            qmix_all[:, qt, j * 512:(j + 1) * 512], pqs[qt][:]
                        )

                # deferred const/table DMAs: behind phase-1a's critical loads
                nc.sync.dma_start(
                    wkv_all[:], wkva_t.ap().rearrange("(t p) c -> p t c", p=P)
                )
                nc.sync.dma_start(
                    ck_t[:], ck_tab.ap().rearrange("(t p) j -> p t j", p=P)
                )
                nc.sync.dma_start(
                    sk_t[:], sk_tab.ap().rearrange("(t p) j -> p t j", p=P)
                )
                nc.sync.dma_start(cq_t[:], cq_tab[:, :])
                nc.sync.dma_start(sq_t[:], sq_tab[:, :])
                nc.sync.dma_start(qbb[:], qb_bias.ap().rearrange("(h p) -> p h", p=P))
                nc.sync.dma_start(qrb[:], qr_bias.ap().rearrange("(a p) -> p a", p=P))
                nc.sync.dma_start(avb[:], av_bias.ap().rearrange("(h p) -> p h", p=P))
                # o_proj weight on the ACT ring (idle until phase 3)
                nc.scalar.dma_start(
                    wo_all[:], wo_t.ap().rearrange("(t p) c -> p t c", p=P)
                )

                for qt in range(NQT):
                    statsq = lnp.tile([P, 3, 6], F32, tag="statsq")
                    for j in range(3):
                        nc.vector.bn_stats(
                            statsq[:, j, :], qmix_all[:, qt, j * 512:(j + 1) * 512]
                        )
                    mvq = lnp.tile([P, 2], F32, tag="mv")
                    nc.vector.bn_aggr(mvq[:], statsq[:])
                    rstdq = lnp.tile([P, 1], F32, tag="rstd")
                    nc.scalar.activation(rstdq[:], mvq[:, 1:2], AF.Sqrt, bias=eps_t[:])
                    nc.vector.reciprocal(rstdq[:], rstdq[:])
                    qln = lnp.tile([P, LAT], BF16, tag="qln")
                    nc.vector.tensor_scalar(
                        qln[:], qmix_all[:, qt, :], mvq[:, 0:1], rstdq[:],
                        op0=OP.subtract, op1=OP.mult,
                    )

                    for lt in range(NLT):
                        ptq = psq.tile([P, P], BF16, tag="ptr", bufs=2)
                        nc.tensor.transpose(ptq[:], qln[:, lt * P:(lt + 1) * P], ident[:])
                        nc.vector.tensor_copy(
                            qlat_t[:, lt, qt * P:(qt + 1) * P], ptq[:]
                        )

            # ============ phase 1b: kv latent + k_rope, LN + RoPE + transpose ========
            with ExitStack() as p1:
                hsl = p1.enter_context(tc.tile_pool(name="hsl", bufs=8))
                lnp = p1.enter_context(tc.tile_pool(name="lnp", bufs=2))
                psk = p1.enter_context(tc.tile_pool(name="psk", bufs=1, space="PSUM"))

                for kt in range(NKT):
                    plat = psk.tile([P, R], F32, tag="plat", bufs=2)
                    prope = psk.tile([P, DR], F32, tag="prope", bufs=2)
                    for dt in range(NDT):
                        hk = hsl.tile([P, P], BF16, tag="hs")
                        nc.sync.dma_start(hk[:], hst4[dt, kt])
                        st = (dt == 0)
                        sp = (dt == NDT - 1)
                        nc.tensor.matmul(
                            plat[:], hk[:], wkv_all[:, dt, 0:R], start=st, stop=sp,
                        )
                        nc.tensor.matmul(
                            prope[:], hk[:], wkv_all[:, dt, R:R + DR],
                            start=st, stop=sp,
                        )

                    # LayerNorm (g/b folded into k_up/v_up) straight off PSUM
                    stats = lnp.tile([P, 6], F32, tag="stats")
                    nc.vector.bn_stats(stats[:], plat[:])
                    mv = lnp.tile([P, 2], F32, tag="mv")
                    nc.vector.bn_aggr(mv[:], stats[:])
                    rstd = lnp.tile([P, 1], F32, tag="rstd")
                    nc.scalar.activation(rstd[:], mv[:, 1:2], AF.Sqrt, bias=eps_t[:])
                    nc.vector.reciprocal(rstd[:], rstd[:])
                    nc.vector.tensor_scalar(
                        kvlat[:, kt, :], plat[:], mv[:, 0:1], rstd[:],
                        op0=OP.subtract, op1=OP.mult,
                    )

                    # RoPE on the k_rope columns, straight off PSUM
                    kro = lnp.tile([P, DR], BF16, tag="kro")
                    t1 = lnp.tile([P, DR // 2], F32, tag="t1")
                    x1 = prope[:, 0:32]
                    x2 = prope[:, 32:64]
                    nc.vector.tensor_tensor(kro[:, 0:32], x1, ck_t[:, kt, :], OP.mult)
                    nc.vector.tensor_tensor(t1[:], x2, sk_t[:, kt, :], OP.mult)
                    nc.vector.tensor_tensor(kro[:, 0:32], kro[:, 0:32], t1[:], OP.subtract)
                    nc.vector.tensor_tensor(kro[:, 32:64], x1, sk_t[:, kt, :], OP.mult)
                    nc.vector.tensor_tensor(t1[:], x2, ck_t[:, kt, :], OP.mult)
                    nc.vector.tensor_tensor(kro[:, 32:64], kro[:, 32:64], t1[:], OP.add)

                    # transpose LN'd latent (4x) and rope (1x) into kfull
                    for j in range(4):
                        pt = psk.tile([P, P], BF16, tag="ptr", bufs=2)
                        nc.tensor.transpose(
                            pt[:], kvlat[:, kt, j * P:(j + 1) * P], ident[:]
                        )
                        nc.vector.tensor_copy(kfull[:, j, kt * P:(kt + 1) * P], pt[:])
                    pt2 = psk.tile([P, P], BF16, tag="ptr", bufs=2)
                    nc.tensor.transpose(pt2[0:DR, :], kro[:], ident[:])
                    nc.vector.tensor_copy(
                        kfull[0:DR, 4, kt * P:(kt + 1) * P], pt2[0:DR, :]
                    )

        # ====================== phase 2: attention head loop ======================
        with ExitStack() as p2:
            p2i = p2.enter_context(ExitStack())
            wqs = p2i.enter_context(tc.tile_pool(name="wqs", bufs=2))
            qwork = p2i.enter_context(tc.tile_pool(name="qwork", bufs=1))
            hwork = p2i.enter_context(tc.tile_pool(name="hwork", bufs=2))
            probs_p = p2i.enter_context(tc.tile_pool(name="probs_p", bufs=2))
            ps2 = p2i.enter_context(tc.tile_pool(name="ps2", bufs=1, space="PSUM"))

            qrh = None
            for h in range(H):
                g, m = divmod(h, 4)
                if m == 0:
                    # ---- RoPE for head group g: q_rope_raw^T then rotate ----
                    qraw = qwork.tile([P, 2, CH], F32, tag="qraw")
                    for half in range(2):
                        wrc = wqs.tile([P, NLT, P], BF16, tag="wq")
                        col0 = half * 512 + g * P
                        nc.sync.dma_start(
                            wrc[:],
                            wqr_t[:, col0:col0 + P].rearrange(
                                "(t p) c -> p t c", p=P
                            ),
                        )
                        pr = ps2.tile([P, 512], F32, tag="small2", bufs=2)
                        for lt in range(NLT):
                            nc.tensor.matmul(
                                pr[:], wrc[:, lt, :], qlat_t[:, lt, :],
                                start=(lt == 0), stop=(lt == NLT - 1),
                            )
                        nc.vector.tensor_scalar(
                            qraw[:, half, :], pr[:],
                            qrb[:, half * 4 + g:half * 4 + g + 1], None, op0=OP.add,
                        )
                    qro = qwork.tile([P, 2, CH], BF16, tag="qro")
                    tm = qwork.tile([P, CH], F32, tag="tm")
                    x1, x2 = qraw[:, 0, :], qraw[:, 1, :]
                    nc.vector.tensor_tensor(tm[:], x2, sq_t[:], OP.mult)
                    nc.vector.tensor_tensor(qro[:, 0, :], x1, cq_t[:], OP.mult)
                    nc.vector.tensor_tensor(qro[:, 0, :], qro[:, 0, :], tm[:], OP.subtract)
                    nc.vector.tensor_tensor(tm[:], x2, cq_t[:], OP.mult)
                    nc.vector.tensor_tensor(qro[:, 1, :], x1, sq_t[:], OP.mult)
                    nc.vector.tensor_tensor(qro[:, 1, :], qro[:, 1, :], tm[:], OP.add)
                    # scatter rope rows for all 4 heads of the group up front
                    qrh = qwork.tile([DR, 4, CH], BF16, tag="qrh")
                    for mm in range(4):
                        nc.sync.dma_start(
                            qrh[0:32, mm, :], qro[mm * 32:(mm + 1) * 32, 0, :]
                        )
                        nc.sync.dma_start(
                            qrh[32:64, mm, :], qro[mm * 32:(mm + 1) * 32, 1, :]
                        )

                # ---- q_nope^T for head h ----
                wb = wqs.tile([P, NLT, P], BF16, tag="wq")
                nc.sync.dma_start(
                    wb[:],
                    wqb_t[:, h * P:(h + 1) * P].rearrange("(t p) c -> p t c", p=P),
                )
                pn = ps2.tile([P, 512], F32, tag="small2", bufs=2)
                for lt in range(NLT):
                    nc.tensor.matmul(
                        pn[:], wb[:, lt, :], qlat_t[:, lt, :],
                        start=(lt == 0), stop=(lt == NLT - 1),
                    )
                qnope = hwork.tile([P, CH], BF16, tag="qnope")
                nc.vector.tensor_scalar(
                    qnope[:], pn[:], qbb[:, h:h + 1], None, op0=OP.add
                )

                # ---- q_abs^T (k_up absorbed) + assemble q_full^T ----
                ku = hwork.tile([P, R], BF16, tag="ku")
                nc.sync.dma_start(ku[:], kup[h * DN:(h + 1) * DN, :])
                qfull = hwork.tile([P, 4, CH], BF16, tag="qfull", bufs=2)
                for rc in range(4):
                    pa = ps2.tile([P, 512], F32, tag="small2", bufs=2)
                    nc.tensor.matmul(
                        pa[:], ku[:, rc * P:(rc + 1) * P], qnope[:],
                        start=True, stop=True,
                    )
                    nc.vector.tensor_copy(qfull[:, rc, :], pa[:])

                # ---- scores^T -> exp -> probs; denominator via ones-matmul ----
                probs = probs_p.tile([P, NKT, CH], BF16, tag="probs")
                psum_d = ps2.tile([P, 512], F32, tag="sum", bufs=1)
                for kt in range(NKT):
                    sc = ps2.tile([P, 512], F32, tag="scores", bufs=2)
                    for j in range(4):
                        nc.tensor.matmul(
                            sc[:], kfull[:, j, kt * P:(kt + 1) * P], qfull[:, j, :],
                            start=(j == 0), stop=False,
                        )
                    nc.tensor.matmul(
                        sc[:], kfull[0:DR, 4, kt * P:(kt + 1) * P],
                        qrh[:, m, :], start=False, stop=True,
                    )
                    nc.scalar.activation(probs[:, kt, :], sc[:], AF.Exp)
                    nc.tensor.matmul(
                        psum_d[:], ones_t[:], probs[:, kt, :],
                        start=(kt == 0), stop=(kt == NKT - 1),
                    )
                recip = hwork.tile([P, CH], F32, tag="recip")
                nc.vector.reciprocal(recip[:], psum_d[:])

                # ---- attn^T = kv_lat-contract(probs), normalized on evict ----
                attnT = hwork.tile([P, 4, CH], BF16, tag="attnT", bufs=1)
                for rc in range(4):
                    pat = ps2.tile([P, 512], F32, tag="attn", bufs=2)
                    for kt in range(NKT):
                        nc.tensor.matmul(
                            pat[:], kvlat[:, kt, rc * P:(rc + 1) * P],
                            probs[:, kt, :],
                            start=(kt == 0), stop=(kt == NKT - 1),
                        )
                    nc.vector.tensor_tensor(
                        attnT[:, rc, :], pat[:], recip[:], OP.mult
                    )

                # ---- attn_v^T[h] = v_up[h]-contract(attn^T) -> resident SBUF ----
                vu = hwork.tile([P, NQT, P], BF16, tag="vu")
                nc.sync.dma_start(vu[:], vup[h])
                pv = ps2.tile([P, 512], F32, tag="sum", bufs=1)
                for rc in range(4):
                    nc.tensor.matmul(
                        pv[:], vu[:, rc, :], attnT[:, rc, :],
                        start=(rc == 0), stop=(rc == 3),
                    )
                nc.vector.tensor_scalar(
                    avn[:, h, :], pv[:], avb[:, h:h + 1], None, op0=OP.add
                )

            p2i.close()

            # =========================== phase 3: o_proj ===========================
            with ExitStack() as p3:
                outp = p3.enter_context(tc.tile_pool(name="outp", bufs=4))
                ps3 = p3.enter_context(tc.tile_pool(name="ps3", bufs=1, space="PSUM"))

                for hd in range(4):
                    po = ps3.tile([P, NQT, 512], F32, tag="po", bufs=2)
                    for kt in range(H):
                        for qc in range(NQT):
                            nc.tensor.matmul(
                                po[:, qc, :],
                                avn[:, kt, qc * P:(qc + 1) * P],
                                wo_all[:, kt, hd * 512:(hd + 1) * 512],
                                start=(kt == 0), stop=(kt == H - 1),
                            )
                    for qc in range(NQT):
                        ot = outp.tile([P, 512], F32, tag="ot")
                        nc.vector.tensor_copy(ot[:], po[:, qc, :])
                        nc.sync.dma_start(
                            out_c[qc * P:(qc + 1) * P, hd * 512:(hd + 1) * 512],
                            ot[:],
                        )

    nc.compile()
    return nc


_NC_CACHE = None


def _get_nc():
    global _NC_CACHE
    if _NC_CACHE is None:
        _NC_CACHE = build_nc()
    return _NC_CACHE


def _prep_in_maps(inputs):
    BF = ml_dtypes.bfloat16
    hidden = np.asarray(inputs["hidden_states"], dtype=np.float32)
    w_qa = np.asarray(inputs["w_qa"], dtype=np.float32)
    ln_qa_g = np.asarray(inputs["ln_qa_g"], dtype=np.float32)
    ln_qa_b = np.asarray(inputs["ln_qa_b"], dtype=np.float32)
    w_qb = np.asarray(inputs["w_qb"], dtype=np.float32)
    w_qrope = np.asarray(inputs["w_qrope"], dtype=np.float32)
    w_kva = np.asarray(inputs["w_kva"], dtype=np.float32)
    ln_kva_g = np.asarray(inputs["ln_kva_g"], dtype=np.float32)
    ln_kva_b = np.asarray(inputs["ln_kva_b"], dtype=np.float32)
    w_kvb = np.asarray(inputs["w_kvb"], dtype=np.float32)
    w_o = np.asarray(inputs["w_o"], dtype=np.float32)
    pos = np.asarray(inputs["position_ids"]).astype(np.int64)

    # host-side prep: layout/transposes, rope tables, LN g/b folding
    hidden_bf = hidden.astype(BF)
    hst4 = [
        np.ascontiguousarray(
            hidden_bf[b].T.reshape(NDT, P, NKT, P).transpose(0, 2, 1, 3)
        )
        for b in range(B)
    ]
    wqa_t = np.ascontiguousarray(w_qa.T.astype(BF))

    # fold ln_qa gain into w_qb / w_qrope columns; bias becomes additive consts
    w_qb_g = w_qb * ln_qa_g[None, :]
    qb_bias = (w_qb @ ln_qa_b).astype(np.float32)                    # [H*DN]
    wqb_t = np.ascontiguousarray(w_qb_g.T.astype(BF))
    w_qr_g = (SCALE * w_qrope) * ln_qa_g[None, :]
    qr_bias_raw = (SCALE * (w_qrope @ ln_qa_b))                      # [H*DR]
    # columns permuted to (half, head, j) ordering to match wqr_t layout
    qr_bias = np.ascontiguousarray(
        qr_bias_raw.reshape(H, 2, DR // 2).transpose(1, 0, 2).reshape(H * DR)
    ).astype(np.float32)
    wqr_t = np.ascontiguousarray(
        w_qr_g.T.reshape(LAT, H, 2, DR // 2).transpose(0, 2, 1, 3)
        .reshape(LAT, H * DR).astype(BF)
    )
    wkva_t = np.ascontiguousarray(w_kva.T.astype(BF))
    # fold ln_kva gain into k_up / v_up; v-side bias uses sum(probs)=1,
    # q-side (scores) bias shift cancels in softmax
    k_up = w_kvb[: H * DN] * ln_kva_g[None, :]
    v_up = w_kvb[H * DN:] * ln_kva_g[None, :]
    av_bias = (w_kvb[H * DN:] @ ln_kva_b).astype(np.float32)         # [H*DV]
    kup_s = np.ascontiguousarray((SCALE * k_up).astype(BF))
    vup_h = np.ascontiguousarray(
        v_up.reshape(H, DV, NQT, P).transpose(0, 3, 2, 1).astype(BF)
    )
    wo_t = np.ascontiguousarray(w_o.T.astype(BF))

    inv_freq = 1.0 / (10000.0 ** (np.arange(0, DR, 2, dtype=np.float64) / DR))
    ang = pos[:, None].astype(np.float64) * inv_freq[None, :]
    cosf = np.ascontiguousarray(np.cos(ang).astype(np.float32))  # [S, 32]
    sinf = np.ascontiguousarray(np.sin(ang).astype(np.float32))

    in_maps = []
    for c in range(N_CORES):
        b, ch = divmod(c, NQT)
        qs = ch * CH
        cq = np.ascontiguousarray(np.tile(cosf[qs:qs + CH, :].T, (NQT, 1)))
        sq = np.ascontiguousarray(np.tile(sinf[qs:qs + CH, :].T, (NQT, 1)))
        hsq4 = np.ascontiguousarray(
            hidden_bf[b, qs:qs + CH, :].reshape(NQT, P, NDT, P).transpose(0, 3, 2, 1)
        )
        in_maps.append({
            "hst4": hst4[b],
            "hsq4": hsq4,
            "wqa_t": wqa_t,
            "wqb_t": wqb_t,
            "wqr_t": wqr_t,
            "wkva_t": wkva_t,
            "kup": kup_s,
            "vup": vup_h,
            "wo_t": wo_t,
            "qb_bias": qb_bias,
            "qr_bias": qr_bias,
            "av_bias": av_bias,
            "ck_tab": cosf,
            "sk_tab": sinf,
            "cq_tab": cq,
            "sq_tab": sq,
        })
    return in_maps


def _assemble_out(res) -> np.ndarray:
    out = np.empty((B, S, D), dtype=np.float32)
    for c in range(N_CORES):
        b, ch = divmod(c, NQT)
        out[b, ch * CH:(ch + 1) * CH, :] = res.results[c]["out_c"]
    return out


def kernel(**inputs) -> np.ndarray:
    nc = _get_nc()
    in_maps = _prep_in_maps(inputs)
    res = run_bass_kernel_spmd(nc, in_maps, core_ids=list(range(N_CORES)))
    return _assemble_out(res)


# revision 13
# speedup vs baseline: 1.0200x; 1.0200x over previous
"""MLA (multi-head latent attention) prefill kernel for 8 Trainium2 NeuronCores.

Sharding: pure data-parallel over (batch, query-chunk). Core c handles batch
c//4 and query rows [512*(c%4), 512*(c%4+1)). Keys/KV-latent (rank 512) are
computed per-core from the full hidden states of the batch (replicated compute,
~9% FLOP overhead) so there are ZERO collectives; every core writes a disjoint
[512, 2048] slice of the output.

All matmuls run in bf16 (full PE rate + fast-weight-load so LDWEIGHTS hides
under the previous matmul's stream), accumulating in fp32 PSUM. LayerNorm
gains/biases are folded into the downstream weights on the host (the q-score
bias shift cancels in softmax; the value-path bias uses sum(probs)=1), so the
device LN is a single (x-mean)*rstd tensor_scalar op applied directly to the
PSUM accumulator. Softmax skips the max-subtraction (logits are O(1): exp
never overflows) and the denominator is an all-ones matmul on the PE. The
o_proj weight is prefetched into SBUF over the scalar-engine DMA ring; attn_v
results stay resident in SBUF (no DRAM round-trip). DMA issue order is tuned
so the first matmul's inputs (hsq, wqa) lead the sync ring.
"""

import math
from contextlib import ExitStack

import ml_dtypes
import numpy as np

import concourse.bass as bass
import concourse.tile as tile
from concourse import bacc, mybir
from concourse.bass_utils import run_bass_kernel_spmd
from concourse.masks import make_identity

F32 = mybir.dt.float32
BF16 = mybir.dt.bfloat16
AF = mybir.ActivationFunctionType
OP = mybir.AluOpType

# problem dims (hardcoded per contest contract)
B, S, D = 2, 2048, 2048
H = 16
LAT = 1536          # Q_LORA
R = 512             # KV_LORA
DN, DR, DV = 128, 64, 128
EPS = 1e-5
SCALE = 1.0 / math.sqrt(DN + DR)

P = 128
CH = 512            # queries per core
NQT = CH // P       # 4 q-tiles per core
NKT = S // P        # 16 key tiles
NDT = D // P        # 16 model-dim tiles
NLT = LAT // P      # 12 latent tiles

N_CORES = 8


def _bcast_rows(t, n, length):
    """[length] DRAM vector -> [n, length] AP with partition step 0."""
    return bass.AP(tensor=t, offset=0, ap=[[0, n], [1, length]])


def build_nc():
    nc = bacc.Bacc(None, target_bir_lowering=False)

    # ---- DRAM I/O (per-core shapes; bf16 where fed to the PE) ----
    hst4 = nc.dram_tensor("hst4", [NDT, NKT, P, P], BF16, kind="ExternalInput")
    hsq4 = nc.dram_tensor("hsq4", [NQT, P, NDT, P], BF16, kind="ExternalInput")
    wqa_t = nc.dram_tensor("wqa_t", [D, LAT], BF16, kind="ExternalInput")
    wqb_t = nc.dram_tensor("wqb_t", [LAT, H * DN], BF16, kind="ExternalInput")
    wqr_t = nc.dram_tensor("wqr_t", [LAT, H * DR], BF16, kind="ExternalInput")
    wkva_t = nc.dram_tensor("wkva_t", [D, R + DR], BF16, kind="ExternalInput")
    kup = nc.dram_tensor("kup", [H * DN, R], BF16, kind="ExternalInput")
    vup = nc.dram_tensor("vup", [H, P, NQT, P], BF16, kind="ExternalInput")
    wo_t = nc.dram_tensor("wo_t", [H * DV, D], BF16, kind="ExternalInput")
    qb_bias = nc.dram_tensor("qb_bias", [H * DN], F32, kind="ExternalInput")
    qr_bias = nc.dram_tensor("qr_bias", [H * DR], F32, kind="ExternalInput")
    av_bias = nc.dram_tensor("av_bias", [H * DV], F32, kind="ExternalInput")
    ck_tab = nc.dram_tensor("ck_tab", [S, DR // 2], F32, kind="ExternalInput")
    sk_tab = nc.dram_tensor("sk_tab", [S, DR // 2], F32, kind="ExternalInput")
    cq_tab = nc.dram_tensor("cq_tab", [P, CH], F32, kind="ExternalInput")
    sq_tab = nc.dram_tensor("sq_tab", [P, CH], F32, kind="ExternalInput")
    out_c = nc.dram_tensor("out_c", [CH, D], F32, kind="ExternalOutput")

    with tile.TileContext(nc) as tc, ExitStack() as octx:
        res = octx.enter_context(tc.tile_pool(name="res", bufs=1))
        # k_full^T: 5 c-subtiles (4x128 latent + 64 rope) x 2048 keys
        kfull = res.tile([P, 5, S], BF16)
        # kv latent, natural layout: [key-part, keytile, R]
        kvlat = res.tile([P, NKT, R], BF16)
        # q latent transposed: [lat-part, lat-tile, q]
        qlat_t = res.tile([P, NLT, CH], BF16)
        # attn_v outputs for all heads, resident until o_proj
        avn = res.tile([P, H, CH], BF16)
        # o_proj weight, resident (DMA issued after phase-1a emission)
        wo_all = res.tile([P, H, D], BF16)

        consts = octx.enter_context(tc.tile_pool(name="consts", bufs=1))
        ident = consts.tile([P, P], BF16)
        make_identity(nc, ident)
        ones_t = consts.tile([P, P], BF16)
        nc.vector.memset(ones_t, 1.0)
        eps_t = consts.tile([P, 1], F32)
        nc.vector.memset(eps_t, EPS)
        cq_t = consts.tile([P, CH], F32)
        sq_t = consts.tile([P, CH], F32)
        qbb = consts.tile([P, H], F32)
        qrb = consts.tile([P, 8], F32)
        avb = consts.tile([P, H], F32)

        with ExitStack() as p01:
            wkvp = p01.enter_context(tc.tile_pool(name="wkvp", bufs=1))
            wkv_all = wkvp.tile([P, NDT, R + DR], BF16)
            gbkv = p01.enter_context(tc.tile_pool(name="gbkv", bufs=1))
            ck_t = gbkv.tile([P, NKT, DR // 2], F32)
            sk_t = gbkv.tile([P, NKT, DR // 2], F32)

            # ================= phase 1a: q latent (LN(hs_q @ w_qa.T))^T =============
            with ExitStack() as p1:
                hsqp = p1.enter_context(tc.tile_pool(name="hsqp", bufs=1))
                wqap = p1.enter_context(tc.tile_pool(name="wqap", bufs=4))
                mixp = p1.enter_context(tc.tile_pool(name="mixp", bufs=1))
                lnp = p1.enter_context(tc.tile_pool(name="lnp", bufs=2))
                psq = p1.enter_context(tc.tile_pool(name="psq", bufs=1, space="PSUM"))

                # hsq first on the sync ring (first matmul's stationary operand)
                hsq_all = hsqp.tile([P, NQT, NDT, P], BF16)
                for qt in range(NQT):
                    nc.sync.dma_start(
                        hsq_all[:, qt], hsq4[qt].rearrange("p d c -> p d c")
                    )
                qmix_all = mixp.tile([P, NQT, LAT], BF16)

                for j in range(3):
                    pqs = []
                    for qt in range(NQT):
                        pq = psq.tile([P, 512], F32, tag=f"pq{qt}", bufs=1, name=f"pq{qt}")
                        pqs.append(pq)
                    for dt in range(NDT):
                        wqa_c = wqap.tile([P, 512], BF16, tag="wqa")
                        nc.sync.dma_start(
                            wqa_c[:],
                            wqa_t[dt * P:(dt + 1) * P, j * 512:(j + 1) * 512],
                        )
                        for qt in range(NQT):
                            nc.tensor.matmul(
                                pqs[qt][:], hsq_all[:, qt, dt, :], wqa_c[:],
                                start=(dt == 0), stop=(dt == NDT - 1),
                            )
                    for qt in range(NQT):
                        nc.vector.tensor_copy(
                            qmix_all[:, qt, j * 512:(j + 1) * 512], pqs[qt][:]
                        )

                # deferred const/table DMAs: behind phase-1a's critical loads
                nc.sync.dma_start(
                    wkv_all[:], wkva_t.ap().rearrange("(t p) c -> p t c", p=P)
                )
                nc.sync.dma_start(
                    ck_t[:], ck_tab.ap().rearrange("(t p) j -> p t j", p=P)
                )
                nc.sync.dma_start(
                    sk_t[:], sk_tab.ap().rearrange("(t p) j -> p t j", p=P)
                )
                nc.sync.dma_start(cq_t[:], cq_tab[:, :])
                nc.sync.dma_start(sq_t[:], sq_tab[:, :])
                nc.sync.dma_start(qbb[:], qb_bias.ap().rearrange("(h p) -> p h", p=P))
                nc.sync.dma_start(qrb[:], qr_bias.ap().rearrange("(a p) -> p a", p=P))
                nc.sync.dma_start(avb[:], av_bias.ap().rearrange("(h p) -> p h", p=P))

                for qt in range(NQT):
                    statsq = lnp.tile([P, 3, 6], F32, tag="statsq")
                    for j in range(3):
                        nc.vector.bn_stats(
                            statsq[:, j, :], qmix_all[:, qt, j * 512:(j + 1) * 512]
                        )
                    mvq = lnp.tile([P, 2], F32, tag="mv")
                    nc.vector.bn_aggr(mvq[:], statsq[:])
                    rstdq = lnp.tile([P, 1], F32, tag="rstd")
                    nc.scalar.activation(rstdq[:], mvq[:, 1:2], AF.Sqrt, bias=eps_t[:])
                    nc.vector.reciprocal(rstdq[:], rstdq[:])
                    qln = lnp.tile([P, LAT], BF16, tag="qln")
                    nc.vector.tensor_scalar(
                        qln[:], qmix_all[:, qt, :], mvq[:, 0:1], rstdq[:],
                        op0=OP.subtract, op1=OP.mult,
                    )

                    for lt in range(NLT):
                        ptq = psq.tile([P, P], BF16, tag="ptr", bufs=2)
                        nc.tensor.transpose(ptq[:], qln[:, lt * P:(lt + 1) * P], ident[:])
                        nc.vector.tensor_copy(
                            qlat_t[:, lt, qt * P:(qt + 1) * P], ptq[:]
                        )

            # ============ phase 1b: kv latent + k_rope, LN + RoPE + transpose ========
            with ExitStack() as p1:
                hsl = p1.enter_context(tc.tile_pool(name="hsl", bufs=8))
                lnp = p1.enter_context(tc.tile_pool(name="lnp", bufs=2))
                psk = p1.enter_context(tc.tile_pool(name="psk", bufs=1, space="PSUM"))

                # o_proj weight on the ACT ring; ACT reaches this only after
                # phase 1a's LN Sqrts, keeping the early sync-ring loads alone
                nc.scalar.dma_start(
                    wo_all[:], wo_t.ap().rearrange("(t p) c -> p t c", p=P)
                )

                def kv_ln_block(kt, plat, prope):
                    # LayerNorm (g/b folded into k_up/v_up) straight off PSUM
                    stats = lnp.tile([P, 6], F32, tag="stats", name="stats")
                    nc.vector.bn_stats(stats[:], plat[:])
                    mv = lnp.tile([P, 2], F32, tag="mv", name="mv")
                    nc.vector.bn_aggr(mv[:], stats[:])
                    rstd = lnp.tile([P, 1], F32, tag="rstd", name="rstd")
                    nc.scalar.activation(rstd[:], mv[:, 1:2], AF.Sqrt, bias=eps_t[:])
                    nc.vector.reciprocal(rstd[:], rstd[:])
                    nc.vector.tensor_scalar(
                        kvlat[:, kt, :], plat[:], mv[:, 0:1], rstd[:],
                        op0=OP.subtract, op1=OP.mult,
                    )

                    # RoPE on the k_rope columns, straight off PSUM
                    kro = lnp.tile([P, DR], BF16, tag="kro", name="kro")
                    t1 = lnp.tile([P, DR // 2], F32, tag="t1", name="t1")
                    x1 = prope[:, 0:32]
                    x2 = prope[:, 32:64]
                    nc.vector.tensor_tensor(kro[:, 0:32], x1, ck_t[:, kt, :], OP.mult)
                    nc.vector.tensor_tensor(t1[:], x2, sk_t[:, kt, :], OP.mult)
                    nc.vector.tensor_tensor(kro[:, 0:32], kro[:, 0:32], t1[:], OP.subtract)
                    nc.vector.tensor_tensor(kro[:, 32:64], x1, sk_t[:, kt, :], OP.mult)
                    nc.vector.tensor_tensor(t1[:], x2, ck_t[:, kt, :], OP.mult)
                    nc.vector.tensor_tensor(kro[:, 32:64], kro[:, 32:64], t1[:], OP.add)

                    # transpose LN'd latent (4x) and rope (1x) into kfull
                    for j in range(4):
                        pt = psk.tile([P, P], BF16, tag="ptr", bufs=2, name="pt")
                        nc.tensor.transpose(
                            pt[:], kvlat[:, kt, j * P:(j + 1) * P], ident[:]
                        )
                        nc.vector.tensor_copy(kfull[:, j, kt * P:(kt + 1) * P], pt[:])
                    pt2 = psk.tile([P, P], BF16, tag="ptr", bufs=2, name="pt2")
                    nc.tensor.transpose(pt2[0:DR, :], kro[:], ident[:])
                    nc.vector.tensor_copy(
                        kfull[0:DR, 4, kt * P:(kt + 1) * P], pt2[0:DR, :]
                    )

                pending = None
                for kt in range(NKT):
                    plat = psk.tile([P, R], F32, tag="plat", bufs=2)
                    prope = psk.tile([P, DR], F32, tag="prope", bufs=2)
                    for dt in range(NDT):
                        hk = hsl.tile([P, P], BF16, tag="hs")
                        nc.sync.dma_start(hk[:], hst4[dt, kt])
                        st = (dt == 0)
                        sp = (dt == NDT - 1)
                        nc.tensor.matmul(
                            plat[:], hk[:], wkv_all[:, dt, 0:R], start=st, stop=sp,
                        )
                        nc.tensor.matmul(
                            prope[:], hk[:], wkv_all[:, dt, R:R + DR],
                            start=st, stop=sp,
                        )
                    # lag-1 software pipeline: kt's LN/rope/transposes are
                    # emitted under kt+1's matmuls so the PE never waits on
                    # the vector LN chain
                    if pending is not None:
                        kv_ln_block(*pending)
                    pending = (kt, plat, prope)
                kv_ln_block(*pending)

        # ====================== phase 2: attention head loop ======================
        with ExitStack() as p2:
            p2i = p2.enter_context(ExitStack())
            wqs = p2i.enter_context(tc.tile_pool(name="wqs", bufs=2))
            qwork = p2i.enter_context(tc.tile_pool(name="qwork", bufs=1))
            hwork = p2i.enter_context(tc.tile_pool(name="hwork", bufs=2))
            probs_p = p2i.enter_context(tc.tile_pool(name="probs_p", bufs=2))
            ps2 = p2i.enter_context(tc.tile_pool(name="ps2", bufs=1, space="PSUM"))

            qrh = None
            for h in range(H):
                g, m = divmod(h, 4)
                if m == 0:
                    # ---- RoPE for head group g: q_rope_raw^T then rotate ----
                    qraw = qwork.tile([P, 2, CH], F32, tag="qraw")
                    for half in range(2):
                        wrc = wqs.tile([P, NLT, P], BF16, tag="wq")
                        col0 = half * 512 + g * P
                        nc.sync.dma_start(
                            wrc[:],
                            wqr_t[:, col0:col0 + P].rearrange(
                                "(t p) c -> p t c", p=P
                            ),
                        )
                        pr = ps2.tile([P, 512], F32, tag="small2", bufs=2)
                        for lt in range(NLT):
                            nc.tensor.matmul(
                                pr[:], wrc[:, lt, :], qlat_t[:, lt, :],
                                start=(lt == 0), stop=(lt == NLT - 1),
                            )
                        nc.vector.tensor_scalar(
                            qraw[:, half, :], pr[:],
                            qrb[:, half * 4 + g:half * 4 + g + 1], None, op0=OP.add,
                        )
                    qro = qwork.tile([P, 2, CH], BF16, tag="qro")
                    tm = qwork.tile([P, CH], F32, tag="tm")
                    x1, x2 = qraw[:, 0, :], qraw[:, 1, :]
                    nc.vector.tensor_tensor(tm[:], x2, sq_t[:], OP.mult)
                    nc.vector.tensor_tensor(qro[:, 0, :], x1, cq_t[:], OP.mult)
                    nc.vector.tensor_tensor(qro[:, 0, :], qro[:, 0, :], tm[:], OP.subtract)
                    nc.vector.tensor_tensor(tm[:], x2, cq_t[:], OP.mult)
                    nc.vector.tensor_tensor(qro[:, 1, :], x1, sq_t[:], OP.mult)
                    nc.vector.tensor_tensor(qro[:, 1, :], qro[:, 1, :], tm[:], OP.add)
                    # scatter rope rows for all 4 heads of the group up front
                    qrh = qwork.tile([DR, 4, CH], BF16, tag="qrh")
                    for mm in range(4):
                        nc.sync.dma_start(
                            qrh[0:32, mm, :], qro[mm * 32:(mm + 1) * 32, 0, :]
                        )
                        nc.sync.dma_start(
                            qrh[32:64, mm, :], qro[mm * 32:(mm + 1) * 32, 1, :]
                        )

                # ---- q_nope^T for head h ----
                wb = wqs.tile([P, NLT, P], BF16, tag="wq")
                nc.sync.dma_start(
                    wb[:],
                    wqb_t[:, h * P:(h + 1) * P].rearrange("(t p) c -> p t c", p=P),
                )
                pn = ps2.tile([P, 512], F32, tag="small2", bufs=2)
                for lt in range(NLT):
                    nc.tensor.matmul(
                        pn[:], wb[:, lt, :], qlat_t[:, lt, :],
                        start=(lt == 0), stop=(lt == NLT - 1),
                    )
                qnope = hwork.tile([P, CH], BF16, tag="qnope")
                nc.vector.tensor_scalar(
                    qnope[:], pn[:], qbb[:, h:h + 1], None, op0=OP.add
                )

                # ---- q_abs^T (k_up absorbed) + assemble q_full^T ----
                ku = hwork.tile([P, R], BF16, tag="ku")
                nc.sync.dma_start(ku[:], kup[h * DN:(h + 1) * DN, :])
                qfull = hwork.tile([P, 4, CH], BF16, tag="qfull", bufs=2)
                for rc in range(4):
                    pa = ps2.tile([P, 512], F32, tag="small2", bufs=2)
                    nc.tensor.matmul(
                        pa[:], ku[:, rc * P:(rc + 1) * P], qnope[:],
                        start=True, stop=True,
                    )
                    nc.vector.tensor_copy(qfull[:, rc, :], pa[:])

                # ---- scores^T -> exp -> probs; denominator via ones-matmul ----
                probs = probs_p.tile([P, NKT, CH], BF16, tag="probs")
                psum_d = ps2.tile([P, 512], F32, tag="sum", bufs=1)
                for kt in range(NKT):
                    sc = ps2.tile([P, 512], F32, tag="scores", bufs=2)
                    for j in range(4):
                        nc.tensor.matmul(
                            sc[:], kfull[:, j, kt * P:(kt + 1) * P], qfull[:, j, :],
                            start=(j == 0), stop=False,
                        )
                    nc.tensor.matmul(
                        sc[:], kfull[0:DR, 4, kt * P:(kt + 1) * P],
                        qrh[:, m, :], start=False, stop=True,
                    )
                    nc.scalar.activation(probs[:, kt, :], sc[:], AF.Exp)
                    nc.tensor.matmul(
                        psum_d[:], ones_t[:], probs[:, kt, :],
                        start=(kt == 0), stop=(kt == NKT - 1),
                    )
                recip = hwork.tile([P, CH], F32, tag="recip")
                nc.vector.reciprocal(recip[:], psum_d[:])

                # ---- attn^T = kv_lat-contract(probs), normalized on evict ----
                attnT = hwork.tile([P, 4, CH], BF16, tag="attnT", bufs=1)
                for rc in range(4):
                    pat = ps2.tile([P, 512], F32, tag="attn", bufs=2)
                    for kt in range(NKT):
                        nc.tensor.matmul(
                            pat[:], kvlat[:, kt, rc * P:(rc + 1) * P],
                            probs[:, kt, :],
                            start=(kt == 0), stop=(kt == NKT - 1),
                        )
                    nc.vector.tensor_tensor(
                        attnT[:, rc, :], pat[:], recip[:], OP.mult
                    )

                # ---- attn_v^T[h] = v_up[h]-contract(attn^T) -> resident SBUF ----
                vu = hwork.tile([P, NQT, P], BF16, tag="vu")
                nc.sync.dma_start(vu[:], vup[h])
                pv = ps2.tile([P, 512], F32, tag="sum", bufs=1)
                for rc in range(4):
                    nc.tensor.matmul(
                        pv[:], vu[:, rc, :], attnT[:, rc, :],
                        start=(rc == 0), stop=(rc == 3),
                    )
                nc.vector.tensor_scalar(
                    avn[:, h, :], pv[:], avb[:, h:h + 1], None, op0=OP.add
                )

            p2i.close()

            # =========================== phase 3: o_proj ===========================
            with ExitStack() as p3:
                outp = p3.enter_context(tc.tile_pool(name="outp", bufs=4))
                ps3 = p3.enter_context(tc.tile_pool(name="ps3", bufs=1, space="PSUM"))

                for hd in range(4):
                    po = ps3.tile([P, NQT, 512], F32, tag="po", bufs=2)
                    for kt in range(H):
                        for qc in range(NQT):
                            nc.tensor.matmul(
                                po[:, qc, :],
                                avn[:, kt, qc * P:(qc + 1) * P],
                                wo_all[:, kt, hd * 512:(hd + 1) * 512],
                                start=(kt == 0), stop=(kt == H - 1),
                            )
                    for qc in range(NQT):
                        ot = outp.tile([P, 512], F32, tag="ot")
                        nc.vector.tensor_copy(ot[:], po[:, qc, :])
                        nc.sync.dma_start(
                            out_c[qc * P:(qc + 1) * P, hd * 512:(hd + 1) * 512],
                            ot[:],
                        )

    nc.compile()
    return nc


_NC_CACHE = None


def _get_nc():
    global _NC_CACHE
    if _NC_CACHE is None:
        _NC_CACHE = build_nc()
    return _NC_CACHE


def _prep_in_maps(inputs):
    BF = ml_dtypes.bfloat16
    hidden = np.asarray(inputs["hidden_states"], dtype=np.float32)
    w_qa = np.asarray(inputs["w_qa"], dtype=np.float32)
    ln_qa_g = np.asarray(inputs["ln_qa_g"], dtype=np.float32)
    ln_qa_b = np.asarray(inputs["ln_qa_b"], dtype=np.float32)
    w_qb = np.asarray(inputs["w_qb"], dtype=np.float32)
    w_qrope = np.asarray(inputs["w_qrope"], dtype=np.float32)
    w_kva = np.asarray(inputs["w_kva"], dtype=np.float32)
    ln_kva_g = np.asarray(inputs["ln_kva_g"], dtype=np.float32)
    ln_kva_b = np.asarray(inputs["ln_kva_b"], dtype=np.float32)
    w_kvb = np.asarray(inputs["w_kvb"], dtype=np.float32)
    w_o = np.asarray(inputs["w_o"], dtype=np.float32)
    pos = np.asarray(inputs["position_ids"]).astype(np.int64)

    # host-side prep: layout/transposes, rope tables, LN g/b folding
    hidden_bf = hidden.astype(BF)
    hst4 = [
        np.ascontiguousarray(
            hidden_bf[b].T.reshape(NDT, P, NKT, P).transpose(0, 2, 1, 3)
        )
        for b in range(B)
    ]
    wqa_t = np.ascontiguousarray(w_qa.T.astype(BF))

    # fold ln_qa gain into w_qb / w_qrope columns; bias becomes additive consts
    w_qb_g = w_qb * ln_qa_g[None, :]
    qb_bias = (w_qb @ ln_qa_b).astype(np.float32)                    # [H*DN]
    wqb_t = np.ascontiguousarray(w_qb_g.T.astype(BF))
    w_qr_g = (SCALE * w_qrope) * ln_qa_g[None, :]
    qr_bias_raw = (SCALE * (w_qrope @ ln_qa_b))                      # [H*DR]
    # columns permuted to (half, head, j) ordering to match wqr_t layout
    qr_bias = np.ascontiguousarray(
        qr_bias_raw.reshape(H, 2, DR // 2).transpose(1, 0, 2).reshape(H * DR)
    ).astype(np.float32)
    wqr_t = np.ascontiguousarray(
        w_qr_g.T.reshape(LAT, H, 2, DR // 2).transpose(0, 2, 1, 3)
        .reshape(LAT, H * DR).astype(BF)
    )
    wkva_t = np.ascontiguousarray(w_kva.T.astype(BF))
    # fold ln_kva gain into k_up / v_up; v-side bias uses sum(probs)=1,
    # q-side (scores) bias shift cancels in softmax
    k_up = w_kvb[: H * DN] * ln_kva_g[None, :]
    v_up = w_kvb[H * DN:] * ln_kva_g[None, :]
    av_bias = (w_kvb[H * DN:] @ ln_kva_b).astype(np.float32)         # [H*DV]
    kup_s = np.ascontiguousarray((SCALE * k_up).astype(BF))
    vup_h = np.ascontiguousarray(
        v_up.reshape(H, DV, NQT, P).transpose(0, 3, 2, 1).astype(BF)
    )
    wo_t = np.ascontiguousarray(w_o.T.astype(BF))

    inv_freq = 1.0 / (10000.0 ** (np.arange(0, DR, 2, dtype=np.float64) / DR))
    ang = pos[:, None].astype(np.float64) * inv_freq[None, :]
    cosf = np.ascontiguousarray(np.cos(ang).astype(np.float32))  # [S, 32]
    sinf = np.ascontiguousarray(np.sin(ang).astype(np.float32))

    in_maps = []
    for c in range(N_CORES):
        b, ch = divmod(c, NQT)
        qs = ch * CH
        cq = np.ascontiguousarray(np.tile(cosf[qs:qs + CH, :].T, (NQT, 1)))
        sq = np.ascontiguousarray(np.tile(sinf[qs:qs + CH, :].T, (NQT, 1)))
        hsq4 = np.ascontiguousarray(
            hidden_bf[b, qs:qs + CH, :].reshape(NQT, P, NDT, P).transpose(0, 3, 2, 1)
        )
        in_maps.append({
            "hst4": hst4[b],
            "hsq4": hsq4,
            "wqa_t": wqa_t,
            "wqb_t": wqb_t,
            "wqr_t": wqr_t,
            "wkva_t": wkva_t,
            "kup": kup_s,
            "vup": vup_h,
            "wo_t": wo_t,
            "qb_bias": qb_bias,
            "qr_bias": qr_bias,
            "av_bias": av_bias,
            "ck_tab": cosf,
            "sk_tab": sinf,
            "cq_tab": cq,
            "sq_tab": sq,
        })
    return in_maps


def _assemble_out(res) -> np.ndarray:
    out = np.empty((B, S, D), dtype=np.float32)
    for c in range(N_CORES):
        b, ch = divmod(c, NQT)
        out[b, ch * CH:(ch + 1) * CH, :] = res.results[c]["out_c"]
    return out


def kernel(**inputs) -> np.ndarray:
    nc = _get_nc()
    in_maps = _prep_in_maps(inputs)
    res = run_bass_kernel_spmd(nc, in_maps, core_ids=list(range(N_CORES)))
    return _assemble_out(res)


# revision 24
# speedup vs baseline: 1.0279x; 1.0077x over previous
"""MLA (multi-head latent attention) prefill kernel for 8 Trainium2 NeuronCores.

Sharding: pure data-parallel over (batch, query-chunk). Core c handles batch
c//4 and query rows [512*(c%4), 512*(c%4+1)). Keys/KV-latent (rank 512) are
computed per-core from the full hidden states of the batch (replicated compute,
~9% FLOP overhead) so there are ZERO collectives; every core writes a disjoint
[512, 2048] slice of the output.

All matmuls run in bf16 (full PE rate + fast-weight-load so LDWEIGHTS hides
under the previous matmul's stream), accumulating in fp32 PSUM. LayerNorm
gains/biases are folded into the downstream weights on the host (the q-score
bias shift cancels in softmax; the value-path bias uses sum(probs)=1), so the
device LN is a single (x-mean)*rstd tensor_scalar op applied directly to the
PSUM accumulator. Softmax skips the max-subtraction (logits are O(1): exp
never overflows) and the denominator is an all-ones matmul on the PE. The
o_proj weight is prefetched into SBUF over the scalar-engine DMA ring; attn_v
results stay resident in SBUF (no DRAM round-trip). DMA issue order is tuned
so the first matmul's inputs (hsq, wqa) lead the sync ring.
"""

import math
from contextlib import ExitStack

import ml_dtypes
import numpy as np

import concourse.bass as bass
import concourse.tile as tile
from concourse import bacc, mybir
from concourse.bass_utils import run_bass_kernel_spmd
from concourse.masks import make_identity

F32 = mybir.dt.float32
BF16 = mybir.dt.bfloat16
AF = mybir.ActivationFunctionType
OP = mybir.AluOpType

# problem dims (hardcoded per contest contract)
B, S, D = 2, 2048, 2048
H = 16
LAT = 1536          # Q_LORA
R = 512             # KV_LORA
DN, DR, DV = 128, 64, 128
EPS = 1e-5
SCALE = 1.0 / math.sqrt(DN + DR)

P = 128
CH = 512            # queries per core
NQT = CH // P       # 4 q-tiles per core
NKT = S // P        # 16 key tiles
NDT = D // P        # 16 model-dim tiles
NLT = LAT // P      # 12 latent tiles

N_CORES = 8


def _bcast_rows(t, n, length):
    """[length] DRAM vector -> [n, length] AP with partition step 0."""
    return bass.AP(tensor=t, offset=0, ap=[[0, n], [1, length]])


def build_nc():
    nc = bacc.Bacc(None, target_bir_lowering=False)

    # ---- DRAM I/O (per-core shapes; bf16 where fed to the PE).
    # All weights are pre-tiled on the host so every DMA is contiguous.
    hst4 = nc.dram_tensor("hst4", [NDT, NKT, P, P], BF16, kind="ExternalInput")
    hsq4 = nc.dram_tensor("hsq4", [NQT, P, NDT, P], BF16, kind="ExternalInput")
    wqa4 = nc.dram_tensor("wqa4", [3, NDT, P, 512], BF16, kind="ExternalInput")
    wqb8 = nc.dram_tensor("wqb8", [H, P, NLT, P], BF16, kind="ExternalInput")
    wqr8 = nc.dram_tensor("wqr8", [8, P, NLT, P], BF16, kind="ExternalInput")
    wkva_r = nc.dram_tensor("wkva_r", [P, NDT, R + DR], BF16, kind="ExternalInput")
    kup = nc.dram_tensor("kup", [H * DN, R], BF16, kind="ExternalInput")
    vup = nc.dram_tensor("vup", [H, P, NQT, P], BF16, kind="ExternalInput")
    wo_r = nc.dram_tensor("wo_r", [P, H, D], BF16, kind="ExternalInput")
    qb_bias = nc.dram_tensor("qb_bias", [H * DN], F32, kind="ExternalInput")
    qr_bias = nc.dram_tensor("qr_bias", [H * DR], F32, kind="ExternalInput")
    av_bias = nc.dram_tensor("av_bias", [H * DV], F32, kind="ExternalInput")
    ck_tab = nc.dram_tensor("ck_tab", [P, NKT, DR // 2], F32, kind="ExternalInput")
    sk_tab = nc.dram_tensor("sk_tab", [P, NKT, DR // 2], F32, kind="ExternalInput")
    cq_tab = nc.dram_tensor("cq_tab", [P, CH], F32, kind="ExternalInput")
    sq_tab = nc.dram_tensor("sq_tab", [P, CH], F32, kind="ExternalInput")
    out_c = nc.dram_tensor("out_c", [CH, D], F32, kind="ExternalOutput")

    with tile.TileContext(nc) as tc, ExitStack() as octx:
        res = octx.enter_context(tc.tile_pool(name="res", bufs=1))
        # k_full^T: 5 c-subtiles (4x128 latent + 64 rope) x 2048 keys
        kfull = res.tile([P, 5, S], BF16)
        # kv latent, natural layout: [key-part, keytile, R]
        kvlat = res.tile([P, NKT, R], BF16)
        # q latent transposed: [lat-part, lat-tile, q]
        qlat_t = res.tile([P, NLT, CH], BF16)
        # attn_v outputs for all heads, resident until o_proj
        avn = res.tile([P, H, CH], BF16)
        # o_proj weight, resident (DMA issued after phase-1a emission)
        wo_all = res.tile([P, H, D], BF16)

        consts = octx.enter_context(tc.tile_pool(name="consts", bufs=1))
        ident = consts.tile([P, P], BF16)
        make_identity(nc, ident)
        ones_t = consts.tile([P, P], BF16)
        nc.vector.memset(ones_t, 1.0)
        eps_t = consts.tile([P, 1], F32)
        nc.vector.memset(eps_t, EPS)
        cq_t = consts.tile([P, CH], F32)
        sq_t = consts.tile([P, CH], F32)
        qbb = consts.tile([P, H], F32)
        qrb = consts.tile([P, 8], F32)
        avb = consts.tile([P, H], F32)

        with ExitStack() as p01:
            wkvp = p01.enter_context(tc.tile_pool(name="wkvp", bufs=1))
            wkv_all = wkvp.tile([P, NDT, R + DR], BF16)
            gbkv = p01.enter_context(tc.tile_pool(name="gbkv", bufs=1))
            ck_t = gbkv.tile([P, NKT, DR // 2], F32)
            sk_t = gbkv.tile([P, NKT, DR // 2], F32)

            # ACT-ring loads: idle ring during phase 1a's sync-ring burst
            nc.scalar.dma_start(wkv_all[:], wkva_r[:, :, :])
            nc.scalar.dma_start(ck_t[:], ck_tab[:, :, :])
            nc.scalar.dma_start(sk_t[:], sk_tab[:, :, :])

            # ================= phase 1a: q latent (LN(hs_q @ w_qa.T))^T =============
            with ExitStack() as p1:
                hsqp = p1.enter_context(tc.tile_pool(name="hsqp", bufs=1))
                wqap = p1.enter_context(tc.tile_pool(name="wqap", bufs=4))
                mixp = p1.enter_context(tc.tile_pool(name="mixp", bufs=1))
                lnp = p1.enter_context(tc.tile_pool(name="lnp", bufs=2))
                psq = p1.enter_context(tc.tile_pool(name="psq", bufs=1, space="PSUM"))

                # hsq first on the sync ring (first matmul's stationary operand)
                hsq_all = hsqp.tile([P, NQT, NDT, P], BF16)
                for qt in range(NQT):
                    nc.sync.dma_start(hsq_all[:, qt], hsq4[qt])
                qmix_all = mixp.tile([P, NQT, LAT], BF16)

                for j in range(3):
                    pqs = []
                    for qt in range(NQT):
                        pq = psq.tile([P, 512], F32, tag=f"pq{qt}", bufs=1, name=f"pq{qt}")
                        pqs.append(pq)
                    for dt in range(NDT):
                        wqa_c = wqap.tile([P, 512], BF16, tag="wqa")
                        nc.sync.dma_start(wqa_c[:], wqa4[j, dt])
                        for qt in range(NQT):
                            nc.tensor.matmul(
                                pqs[qt][:], hsq_all[:, qt, dt, :], wqa_c[:],
                                start=(dt == 0), stop=(dt == NDT - 1),
                            )
                    for qt in range(NQT):
                        nc.vector.tensor_copy(
                            qmix_all[:, qt, j * 512:(j + 1) * 512], pqs[qt][:]
                        )

                # deferred const/table DMAs: behind phase-1a's critical loads
                nc.sync.dma_start(cq_t[:], cq_tab[:, :])
                nc.sync.dma_start(sq_t[:], sq_tab[:, :])
                nc.sync.dma_start(qbb[:], qb_bias.ap().rearrange("(h p) -> p h", p=P))
                nc.sync.dma_start(qrb[:], qr_bias.ap().rearrange("(a p) -> p a", p=P))
                nc.sync.dma_start(avb[:], av_bias.ap().rearrange("(h p) -> p h", p=P))

                for qt in range(NQT):
                    statsq = lnp.tile([P, 3, 6], F32, tag="statsq")
                    for j in range(3):
                        nc.vector.bn_stats(
                            statsq[:, j, :], qmix_all[:, qt, j * 512:(j + 1) * 512]
                        )
                    mvq = lnp.tile([P, 2], F32, tag="mv")
                    nc.vector.bn_aggr(mvq[:], statsq[:])
                    rstdq = lnp.tile([P, 1], F32, tag="rstd")
                    nc.scalar.activation(rstdq[:], mvq[:, 1:2], AF.Sqrt, bias=eps_t[:])
                    nc.vector.reciprocal(rstdq[:], rstdq[:])
                    qln = lnp.tile([P, LAT], BF16, tag="qln")
                    nc.vector.tensor_scalar(
                        qln[:], qmix_all[:, qt, :], mvq[:, 0:1], rstdq[:],
                        op0=OP.subtract, op1=OP.mult,
                    )

                    for lt in range(NLT):
                        ptq = psq.tile([P, P], BF16, tag="ptr", bufs=2)
                        nc.tensor.transpose(ptq[:], qln[:, lt * P:(lt + 1) * P], ident[:])
                        nc.vector.tensor_copy(
                            qlat_t[:, lt, qt * P:(qt + 1) * P], ptq[:]
                        )

            # ============ phase 1b: kv latent + k_rope, LN + RoPE + transpose ========
            with ExitStack() as p1:
                hsl = p1.enter_context(tc.tile_pool(name="hsl", bufs=8))
                lnp = p1.enter_context(tc.tile_pool(name="lnp", bufs=2))
                psk = p1.enter_context(tc.tile_pool(name="psk", bufs=1, space="PSUM"))

                # o_proj weight on the ACT ring; ACT reaches this only after
                # phase 1a's LN Sqrts, keeping the early sync-ring loads alone
                nc.scalar.dma_start(wo_all[:], wo_r[:, :, :])

                def kv_ln_block(kt, plat, prope):
                    # LayerNorm (g/b folded into k_up/v_up) straight off PSUM
                    stats = lnp.tile([P, 6], F32, tag="stats", name="stats")
                    nc.vector.bn_stats(stats[:], plat[:])
                    mv = lnp.tile([P, 2], F32, tag="mv", name="mv")
                    nc.vector.bn_aggr(mv[:], stats[:])
                    rstd = lnp.tile([P, 1], F32, tag="rstd", name="rstd")
                    nc.scalar.activation(rstd[:], mv[:, 1:2], AF.Sqrt, bias=eps_t[:])
                    nc.vector.reciprocal(rstd[:], rstd[:])
                    nc.vector.tensor_scalar(
                        kvlat[:, kt, :], plat[:], mv[:, 0:1], rstd[:],
                        op0=OP.subtract, op1=OP.mult,
                    )

                    # RoPE on the k_rope columns, straight off PSUM
                    kro = lnp.tile([P, DR], BF16, tag="kro", name="kro")
                    t1 = lnp.tile([P, DR // 2], F32, tag="t1", name="t1")
                    x1 = prope[:, 0:32]
                    x2 = prope[:, 32:64]
                    nc.vector.tensor_tensor(kro[:, 0:32], x1, ck_t[:, kt, :], OP.mult)
                    nc.vector.tensor_tensor(t1[:], x2, sk_t[:, kt, :], OP.mult)
                    nc.vector.tensor_tensor(kro[:, 0:32], kro[:, 0:32], t1[:], OP.subtract)
                    nc.vector.tensor_tensor(kro[:, 32:64], x1, sk_t[:, kt, :], OP.mult)
                    nc.vector.tensor_tensor(t1[:], x2, ck_t[:, kt, :], OP.mult)
                    nc.vector.tensor_tensor(kro[:, 32:64], kro[:, 32:64], t1[:], OP.add)

                    # transpose LN'd latent (4x) and rope (1x) into kfull
                    for j in range(4):
                        pt = psk.tile([P, P], BF16, tag="ptr", bufs=2, name="pt")
                        nc.tensor.transpose(
                            pt[:], kvlat[:, kt, j * P:(j + 1) * P], ident[:]
                        )
                        nc.vector.tensor_copy(kfull[:, j, kt * P:(kt + 1) * P], pt[:])
                    pt2 = psk.tile([P, P], BF16, tag="ptr", bufs=2, name="pt2")
                    nc.tensor.transpose(pt2[0:DR, :], kro[:], ident[:])
                    nc.vector.tensor_copy(
                        kfull[0:DR, 4, kt * P:(kt + 1) * P], pt2[0:DR, :]
                    )

                pending = None
                for kt in range(NKT):
                    plat = psk.tile([P, R], F32, tag="plat", bufs=2)
                    prope = psk.tile([P, DR], F32, tag="prope", bufs=2)
                    for dt in range(NDT):
                        hk = hsl.tile([P, P], BF16, tag="hs")
                        nc.sync.dma_start(hk[:], hst4[dt, kt])
                        st = (dt == 0)
                        sp = (dt == NDT - 1)
                        nc.tensor.matmul(
                            plat[:], hk[:], wkv_all[:, dt, 0:R], start=st, stop=sp,
                        )
                        nc.tensor.matmul(
                            prope[:], hk[:], wkv_all[:, dt, R:R + DR],
                            start=st, stop=sp,
                        )
                    # lag-1 software pipeline: kt's LN/rope/transposes are
                    # emitted under kt+1's matmuls so the PE never waits on
                    # the vector LN chain
                    if pending is not None:
                        kv_ln_block(*pending)
                    pending = (kt, plat, prope)
                kv_ln_block(*pending)

        # ====================== phase 2: attention head loop ======================
        with ExitStack() as p2:
            p2i = p2.enter_context(ExitStack())
            wqs = p2i.enter_context(tc.tile_pool(name="wqs", bufs=2))
            qwork = p2i.enter_context(tc.tile_pool(name="qwork", bufs=1))
            hwork = p2i.enter_context(tc.tile_pool(name="hwork", bufs=2))
            probs_p = p2i.enter_context(tc.tile_pool(name="probs_p", bufs=2))
            ps2 = p2i.enter_context(tc.tile_pool(name="ps2", bufs=1, space="PSUM"))

            qrh = None
            for h in range(H):
                g, m = divmod(h, 4)
                if m == 0:
                    # ---- RoPE for head group g: q_rope_raw^T then rotate ----
                    qraw = qwork.tile([P, 2, CH], F32, tag="qraw")
                    for half in range(2):
                        wrc = wqs.tile([P, NLT, P], BF16, tag="wq")
                        nc.scalar.dma_start(wrc[:], wqr8[half * 4 + g])
                        pr = ps2.tile([P, 512], F32, tag="small2", bufs=2)
                        for lt in range(NLT):
                            nc.tensor.matmul(
                                pr[:], wrc[:, lt, :], qlat_t[:, lt, :],
                                start=(lt == 0), stop=(lt == NLT - 1),
                            )
                        nc.vector.tensor_scalar(
                            qraw[:, half, :], pr[:],
                            qrb[:, half * 4 + g:half * 4 + g + 1], None, op0=OP.add,
                        )
                    qro = qwork.tile([P, 2, CH], BF16, tag="qro")
                    tm = qwork.tile([P, CH], F32, tag="tm")
                    x1, x2 = qraw[:, 0, :], qraw[:, 1, :]
                    nc.vector.tensor_tensor(tm[:], x2, sq_t[:], OP.mult)
                    nc.vector.tensor_tensor(qro[:, 0, :], x1, cq_t[:], OP.mult)
                    nc.vector.tensor_tensor(qro[:, 0, :], qro[:, 0, :], tm[:], OP.subtract)
                    nc.vector.tensor_tensor(tm[:], x2, cq_t[:], OP.mult)
                    nc.vector.tensor_tensor(qro[:, 1, :], x1, sq_t[:], OP.mult)
                    nc.vector.tensor_tensor(qro[:, 1, :], qro[:, 1, :], tm[:], OP.add)
                    # scatter rope rows for all 4 heads of the group up front
                    qrh = qwork.tile([DR, 4, CH], BF16, tag="qrh")
                    for mm in range(4):
                        nc.sync.dma_start(
                            qrh[0:32, mm, :], qro[mm * 32:(mm + 1) * 32, 0, :]
                        )
                        nc.sync.dma_start(
                            qrh[32:64, mm, :], qro[mm * 32:(mm + 1) * 32, 1, :]
                        )

                # ---- q_nope^T for head h ----
                wb = wqs.tile([P, NLT, P], BF16, tag="wq")
                nc.scalar.dma_start(wb[:], wqb8[h])
                pn = ps2.tile([P, 512], F32, tag="small2", bufs=2)
                for lt in range(NLT):
                    nc.tensor.matmul(
                        pn[:], wb[:, lt, :], qlat_t[:, lt, :],
                        start=(lt == 0), stop=(lt == NLT - 1),
                    )
                qnope = hwork.tile([P, CH], BF16, tag="qnope")
                nc.vector.tensor_scalar(
                    qnope[:], pn[:], qbb[:, h:h + 1], None, op0=OP.add
                )

                # ---- q_abs^T (k_up absorbed) + assemble q_full^T ----
                ku = hwork.tile([P, R], BF16, tag="ku")
                nc.sync.dma_start(ku[:], kup[h * DN:(h + 1) * DN, :])
                qfull = hwork.tile([P, 4, CH], BF16, tag="qfull", bufs=2)
                for rc in range(4):
                    pa = ps2.tile([P, 512], F32, tag="small2", bufs=2)
                    nc.tensor.matmul(
                        pa[:], ku[:, rc * P:(rc + 1) * P], qnope[:],
                        start=True, stop=True,
                    )
                    nc.vector.tensor_copy(qfull[:, rc, :], pa[:])

                # ---- scores^T -> exp -> probs; denominator via ones-matmul ----
                probs = probs_p.tile([P, NKT, CH], BF16, tag="probs")
                psum_d = ps2.tile([P, 512], F32, tag="sum", bufs=1)
                for kt in range(NKT):
                    sc = ps2.tile([P, 512], F32, tag="scores", bufs=2)
                    for j in range(4):
                        nc.tensor.matmul(
                            sc[:], kfull[:, j, kt * P:(kt + 1) * P], qfull[:, j, :],
                            start=(j == 0), stop=False,
                        )
                    nc.tensor.matmul(
                        sc[:], kfull[0:DR, 4, kt * P:(kt + 1) * P],
                        qrh[:, m, :], start=False, stop=True,
                    )
                    nc.scalar.activation(probs[:, kt, :], sc[:], AF.Exp)
                    nc.tensor.matmul(
                        psum_d[:], ones_t[:], probs[:, kt, :],
                        start=(kt == 0), stop=(kt == NKT - 1),
                    )
                recip = hwork.tile([P, CH], F32, tag="recip")
                nc.vector.reciprocal(recip[:], psum_d[:])

                # ---- attn^T = kv_lat-contract(probs), normalized on evict ----
                attnT = hwork.tile([P, 4, CH], BF16, tag="attnT", bufs=1)
                for rc in range(4):
                    pat = ps2.tile([P, 512], F32, tag="attn", bufs=2)
                    for kt in range(NKT):
                        nc.tensor.matmul(
                            pat[:], kvlat[:, kt, rc * P:(rc + 1) * P],
                            probs[:, kt, :],
                            start=(kt == 0), stop=(kt == NKT - 1),
                        )
                    nc.vector.tensor_tensor(
                        attnT[:, rc, :], pat[:], recip[:], OP.mult
                    )

                # ---- attn_v^T[h] = v_up[h]-contract(attn^T) -> resident SBUF ----
                vu = hwork.tile([P, NQT, P], BF16, tag="vu")
                nc.sync.dma_start(vu[:], vup[h])
                pv = ps2.tile([P, 512], F32, tag="sum", bufs=1)
                for rc in range(4):
                    nc.tensor.matmul(
                        pv[:], vu[:, rc, :], attnT[:, rc, :],
                        start=(rc == 0), stop=(rc == 3),
                    )
                nc.vector.tensor_scalar(
                    avn[:, h, :], pv[:], avb[:, h:h + 1], None, op0=OP.add
                )

            p2i.close()

            # =========================== phase 3: o_proj ===========================
            with ExitStack() as p3:
                outp = p3.enter_context(tc.tile_pool(name="outp", bufs=4))
                ps3 = p3.enter_context(tc.tile_pool(name="ps3", bufs=1, space="PSUM"))

                for hd in range(4):
                    po = ps3.tile([P, NQT, 512], F32, tag="po", bufs=2)
                    for kt in range(H):
                        for qc in range(NQT):
                            nc.tensor.matmul(
                                po[:, qc, :],
                                avn[:, kt, qc * P:(qc + 1) * P],
                                wo_all[:, kt, hd * 512:(hd + 1) * 512],
                                start=(kt == 0), stop=(kt == H - 1),
                            )
                    for qc in range(NQT):
                        ot = outp.tile([P, 512], F32, tag="ot")
                        nc.vector.tensor_copy(ot[:], po[:, qc, :])
                        nc.sync.dma_start(
                            out_c[qc * P:(qc + 1) * P, hd * 512:(hd + 1) * 512],
                            ot[:],
                        )

    nc.compile()
    return nc


_NC_CACHE = None


def _get_nc():
    global _NC_CACHE
    if _NC_CACHE is None:
        _NC_CACHE = build_nc()
    return _NC_CACHE


def _prep_in_maps(inputs):
    BF = ml_dtypes.bfloat16
    hidden = np.asarray(inputs["hidden_states"], dtype=np.float32)
    w_qa = np.asarray(inputs["w_qa"], dtype=np.float32)
    ln_qa_g = np.asarray(inputs["ln_qa_g"], dtype=np.float32)
    ln_qa_b = np.asarray(inputs["ln_qa_b"], dtype=np.float32)
    w_qb = np.asarray(inputs["w_qb"], dtype=np.float32)
    w_qrope = np.asarray(inputs["w_qrope"], dtype=np.float32)
    w_kva = np.asarray(inputs["w_kva"], dtype=np.float32)
    ln_kva_g = np.asarray(inputs["ln_kva_g"], dtype=np.float32)
    ln_kva_b = np.asarray(inputs["ln_kva_b"], dtype=np.float32)
    w_kvb = np.asarray(inputs["w_kvb"], dtype=np.float32)
    w_o = np.asarray(inputs["w_o"], dtype=np.float32)
    pos = np.asarray(inputs["position_ids"]).astype(np.int64)

    # host-side prep: layout/transposes, rope tables, LN g/b folding
    hidden_bf = hidden.astype(BF)
    hst4 = [
        np.ascontiguousarray(
            hidden_bf[b].T.reshape(NDT, P, NKT, P).transpose(0, 2, 1, 3)
        )
        for b in range(B)
    ]
    # wqa pre-tiled [3, NDT, P, 512] so each (j, dt) chunk DMA is contiguous
    wqa4 = np.ascontiguousarray(
        w_qa.T.reshape(NDT, P, 3, 512).transpose(2, 0, 1, 3).astype(BF)
    )

    # fold ln_qa gain into w_qb / w_qrope columns; bias becomes additive consts
    w_qb_g = w_qb * ln_qa_g[None, :]
    qb_bias = (w_qb @ ln_qa_b).astype(np.float32)                    # [H*DN]
    # per-head [P(lat-sub), NLT, P(out)] contiguous blocks
    wqb8 = np.ascontiguousarray(
        w_qb_g.T.reshape(NLT, P, H, P).transpose(2, 1, 0, 3).astype(BF)
    )
    w_qr_g = (SCALE * w_qrope) * ln_qa_g[None, :]
    qr_bias_raw = (SCALE * (w_qrope @ ln_qa_b))                      # [H*DR]
    # columns permuted to (half, head, j) ordering to match wqr8 layout
    qr_bias = np.ascontiguousarray(
        qr_bias_raw.reshape(H, 2, DR // 2).transpose(1, 0, 2).reshape(H * DR)
    ).astype(np.float32)
    # [8 (half*4+g), P(lat-sub), NLT, P(out=4 heads x 32)] contiguous
    wqr_cols = (
        w_qr_g.T.reshape(LAT, H, 2, DR // 2).transpose(0, 2, 1, 3)
        .reshape(LAT, H * DR)
    )
    wqr8 = np.ascontiguousarray(
        wqr_cols.reshape(NLT, P, 8, P).transpose(2, 1, 0, 3).astype(BF)
    )
    wkva_r = np.ascontiguousarray(
        w_kva.T.reshape(NDT, P, R + DR).transpose(1, 0, 2).astype(BF)
    )
    # fold ln_kva gain into k_up / v_up; v-side bias uses sum(probs)=1,
    # q-side (scores) bias shift cancels in softmax
    k_up = w_kvb[: H * DN] * ln_kva_g[None, :]
    v_up = w_kvb[H * DN:] * ln_kva_g[None, :]
    av_bias = (w_kvb[H * DN:] @ ln_kva_b).astype(np.float32)         # [H*DV]
    kup_s = np.ascontiguousarray((SCALE * k_up).astype(BF))
    vup_h = np.ascontiguousarray(
        v_up.reshape(H, DV, NQT, P).transpose(0, 3, 2, 1).astype(BF)
    )
    wo_r = np.ascontiguousarray(
        w_o.T.reshape(H, P, D).transpose(1, 0, 2).astype(BF)
    )

    inv_freq = 1.0 / (10000.0 ** (np.arange(0, DR, 2, dtype=np.float64) / DR))
    ang = pos[:, None].astype(np.float64) * inv_freq[None, :]
    cosf = np.ascontiguousarray(np.cos(ang).astype(np.float32))  # [S, 32]
    sinf = np.ascontiguousarray(np.sin(ang).astype(np.float32))
    ck_r = np.ascontiguousarray(cosf.reshape(NKT, P, DR // 2).transpose(1, 0, 2))
    sk_r = np.ascontiguousarray(sinf.reshape(NKT, P, DR // 2).transpose(1, 0, 2))

    in_maps = []
    for c in range(N_CORES):
        b, ch = divmod(c, NQT)
        qs = ch * CH
        cq = np.ascontiguousarray(np.tile(cosf[qs:qs + CH, :].T, (NQT, 1)))
        sq = np.ascontiguousarray(np.tile(sinf[qs:qs + CH, :].T, (NQT, 1)))
        hsq4 = np.ascontiguousarray(
            hidden_bf[b, qs:qs + CH, :].reshape(NQT, P, NDT, P).transpose(0, 3, 2, 1)
        )
        in_maps.append({
            "hst4": hst4[b],
            "hsq4": hsq4,
            "wqa4": wqa4,
            "wqb8": wqb8,
            "wqr8": wqr8,
            "wkva_r": wkva_r,
            "kup": kup_s,
            "vup": vup_h,
            "wo_r": wo_r,
            "qb_bias": qb_bias,
            "qr_bias": qr_bias,
            "av_bias": av_bias,
            "ck_tab": ck_r,
            "sk_tab": sk_r,
            "cq_tab": cq,
            "sq_tab": sq,
        })
    return in_maps


def _assemble_out(res) -> np.ndarray:
    out = np.empty((B, S, D), dtype=np.float32)
    for c in range(N_CORES):
        b, ch = divmod(c, NQT)
        out[b, ch * CH:(ch + 1) * CH, :] = res.results[c]["out_c"]
    return out


def kernel(**inputs) -> np.ndarray:
    nc = _get_nc()
    in_maps = _prep_in_maps(inputs)
    res = run_bass_kernel_spmd(nc, in_maps, core_ids=list(range(N_CORES)))
    return _assemble_out(res)
